# revision 22
# baseline (speedup 1.0000x reference)
"""CondensedLinearFineGrained on 8 TRN2 NeuronCores.

Math: out[b,o] = sum_k W[o,k] * input[b, mask[o,k]] + bias[o]
with B=256, IN_F=4096, OUT_F=4096, K=256.

Strategy
--------
Reformulate as a dense matmul:  out = input @ A^T + bias  where
A[o,f] = sum_{k: mask[o,k]==f} W[o,k]  (duplicates within a row are summed).
The dense reformulation does 16x the MACs of the gather form but runs on the
128x128 systolic array (~13.8us/core bf16); the gather form would push 268M
elements through vector/gpsimd (hundreds of us).

Sharding: output neurons, 512 per core. Per core the A^T f-tiles
[128f x 512o] bf16 come from two producers running in parallel:
  - N_DENSE tiles pre-densified on the host, bulk-DMA'd (131KB/tile).
  - NPAIR*2 tiles built on-device by gpsimd local_scatter from host-packed
    CSC (per-feature (o,weight) lists, deduped, -1-padded, int16 idx),
    two tiles per scatter (~1.34us/pair, ~51KB CSC per pair).
The split is chosen so the DMA stream (~335GB/s measured) and the gpsimd
scatter pipeline both finish just under the PE's own span (64 matmuls of
[128bx128f]@[128f x 512o] ~= 216ns each warm).

Schedule notes (from perfetto traces):
  - exec_time is measured from the first engine instruction to the last
    sequencer drain, so the tail matters as much as the stream: semaphores
    are recycled inline on their waiter engines (no second block), psum0 is
    finished STAG tiles early so its copy+out-DMA overlap the remaining
    matmuls, and copy of psum1 is split across DVE and Act.
  - The PE HAM clock gate needs ~3.4us of *continuous* PE busy time to lift
    the 1.2GHz->2.4GHz throttle, so the PE stream opens with back-to-back
    warmup matmuls on uninitialized SBUF with no semaphore waits, sized to
    bridge until the first real tile's data lands.
  - Tiles are consumed in an order that interleaves scattered pairs (gpsimd
    paced, 1.34us/pair) with dense tiles so neither producer stalls the PE;
    the host permutes inT/atd into consumption order so DMA chunks stay
    contiguous.
"""

import numpy as np
import ml_dtypes

B = 256
IN_F = 4096
OUT_F = 4096
K = 256
N_CORES = 8
O_SH = OUT_F // N_CORES  # 512 output rows per core
NT = IN_F // 128         # 32 feature tiles
NB = B // 128            # 2 batch tiles

NPAIR = 8        # scatter pairs (2 f-tiles each) built on-device
N_WARM = 5       # back-to-back PE warmups to lift the HAM clock gate
STAG = 4         # psum0 finishes STAG tiles before psum1 (tail overlap)

_BF16 = ml_dtypes.bfloat16

_prog_cache = {}


def _schedule(npair: int):
    """Consumption-order schedule shared by host packing and the program.

    Returns (korder, kinfo) where korder[k] = original f-tile id consumed
    at step k, and kinfo[k] = ('d', atd_slot) or ('s', pair, half).
    Dense tiles are orig f-tiles [0, nd); pair j is orig tiles
    (nd+2j, nd+2j+1). Order: 2 dense up front, then each pair followed by
    one dense filler (pair production is slower than PE consumption), then
    the remaining dense tiles.
    """
    nd = NT - 2 * npair
    korder, kinfo = [], []
    slot = 0

    def dense(n):
        nonlocal slot
        for _ in range(n):
            if slot < nd:
                korder.append(slot)
                kinfo.append(("d", slot))
                slot += 1

    dense(2)
    for j in range(npair):
        korder.append(nd + 2 * j)
        kinfo.append(("s", j, 0))
        korder.append(nd + 2 * j + 1)
        kinfo.append(("s", j, 1))
        dense(1)
    dense(nd)
    assert len(korder) == NT
    return korder, kinfo


def _chunk_plan(npair: int):
    """(inT chunks, atd chunks, csc chunks) as lists of (start, end)."""
    nd = NT - 2 * npair

    def chunks(total, sizes):
        out, p, i = [], 0, 0
        while p < total:
            s = min(sizes[min(i, len(sizes) - 1)], total - p)
            out.append((p, p + s))
            p += s
            i += 1
        return out

    # Each dma_start costs ~0.7us of serialized descriptor-gen on its
    # issuing sequencer, so chunks are few and grow toward the tail.
    in_chunks = chunks(NT, [3, 5, 6, 7, 7])
    atd_chunks = chunks(nd, [2, 2, 2, 2, 2, 3])
    csc_chunks = [(0, npair)]
    return in_chunks, atd_chunks, csc_chunks


def _build_program_raw(wpad: int, npair: int):
    """Hand-scheduled SPMD program: explicit per-engine streams + semaphores."""
    from contextlib import ExitStack
    from concourse import bacc, mybir, library_config

    nd = NT - 2 * npair
    korder, kinfo = _schedule(npair)
    in_chunks, atd_chunks, csc_chunks = _chunk_plan(npair)

    def chunk_of(chunks, t):
        for c, (c0, c1) in enumerate(chunks):
            if c0 <= t < c1:
                return c
        raise AssertionError

    nc = bacc.Bacc("TRN2", target_bir_lowering=False, debug=False)
    dt = mybir.dt

    inT_d = nc.dram_tensor("inT", [128, NT, B], dt.bfloat16, kind="ExternalInput")
    bias_d = nc.dram_tensor("bias", [1, O_SH], dt.bfloat16, kind="ExternalInput")
    if npair:
        idx_d = nc.dram_tensor("cscidx", [128, npair, wpad], dt.int16,
                               kind="ExternalInput")
        val_d = nc.dram_tensor("cscval", [128, npair, wpad], dt.bfloat16,
                               kind="ExternalInput")
    if nd:
        atd_d = nc.dram_tensor("atd", [128, nd, O_SH], dt.bfloat16,
                               kind="ExternalInput")
    out_d = nc.dram_tensor("out", [NB, 128, O_SH], dt.float32,
                           kind="ExternalOutput")

    inT_sb = nc.alloc_sbuf_tensor("inT_sb", [128, NT, B], dt.bfloat16).ap()
    bias_sb = nc.alloc_sbuf_tensor("bias_sb", [1, O_SH], dt.bfloat16).ap()
    ones_sb = nc.alloc_sbuf_tensor("ones_sb", [1, 128], dt.bfloat16).ap()
    warm_sb = nc.alloc_sbuf_tensor("warm_sb", [128, 128 + O_SH],
                                   dt.bfloat16).ap()
    if npair:
        idx_sb = nc.alloc_sbuf_tensor("idx_sb", [128, npair, wpad],
                                      dt.int16).ap()
        val_sb = nc.alloc_sbuf_tensor("val_sb", [128, npair, wpad],
                                      dt.bfloat16).ap()
        at_sb = nc.alloc_sbuf_tensor("at_sb", [128, npair, 2, O_SH],
                                     dt.bfloat16).ap()
    if nd:
        atd_sb = nc.alloc_sbuf_tensor("atd_sb", [128, nd, O_SH],
                                      dt.bfloat16).ap()
    outs_sb = [nc.alloc_sbuf_tensor(f"out_sb{i}", [128, O_SH], dt.float32).ap()
               for i in range(NB)]

    psums = [nc.alloc_psum_tensor(f"ps{i}", [128, O_SH], dt.float32).ap()
             for i in range(NB)]
    ps_warm = nc.alloc_psum_tensor("ps_warm", [128, O_SH], dt.float32).ap()

    with ExitStack() as ctx:
        sem = lambda name: ctx.enter_context(nc.semaphore(name))
        # One semaphore per DMA: sub-transfers of back-to-back DMAs on one
        # queue can complete out of order, so prefix thresholds on a shared
        # semaphore would be unsound.
        s_bias = sem("s_bias")
        s_in = [sem(f"s_in{c}") for c in range(len(in_chunks))]
        s_atd = [sem(f"s_atd{c}") for c in range(len(atd_chunks))] if nd else []
        s_ci = [sem(f"s_ci{c}") for c in range(len(csc_chunks))] if npair else []
        s_cv = [sem(f"s_cv{c}") for c in range(len(csc_chunks))] if npair else []
        # out-DMA completion sems: incremented (the BIR verifier requires a
        # sem update on every DMA) but never waited in-kernel — the
        # runtime's queue drain covers output completion. Never cleared:
        # nothing depends on their value.
        s_od = [sem(f"s_od{i}") for i in range(NB)]
        s_g = sem("s_g")      # scatter pairs published
        s_v = sem("s_v")      # ones_sb ready
        s_w = sem("s_w")      # warm_sb ready
        s_ps = sem("s_ps")    # PE accumulation done per psum
        s_cp0 = sem("s_cp0")  # psum0->sbuf copy done
        s_c1v = sem("s_c1v")  # psum1 copy done

        with nc.Block() as block:

            # The input feed is interleaved across the two HWDGE rings in
            # PE/gpsimd consumption order, greedily byte-balanced so neither
            # ring starves a consumer. CSC chunks ride early (gpsimd's
            # scatter pipeline is the slowest producer); atd/inT chunks
            # follow the tile consumption order (host pre-permuted).
            feed = []  # (dst, src, sem, bytes)

            def add(dst, src, s, nbytes):
                feed.append((dst, src, s, nbytes))

            ic = lambda c: in_chunks[c]
            ac = lambda c: atd_chunks[c]
            cc = lambda c: csc_chunks[c]

            add(bias_sb[:], bias_d[:], s_bias, O_SH * 2)
            for c in range(max(len(in_chunks), len(atd_chunks))):
                if nd and c < len(atd_chunks):
                    c0, c1 = ac(c)
                    add(atd_sb[:, c0:c1, :], atd_d[:, c0:c1, :], s_atd[c],
                        (c1 - c0) * O_SH * 2)
                if c < len(in_chunks):
                    c0, c1 = ic(c)
                    add(inT_sb[:, c0:c1, :], inT_d[:, c0:c1, :], s_in[c],
                        (c1 - c0) * B * 2)

            qa, qb, ba, bb = [], [], 0, 0
            for dst, src, s, w in feed:
                if ba <= bb:
                    qa.append((dst, src, s)); ba += w
                else:
                    qb.append((dst, src, s)); bb += w

            @block.sync
            def _(sy):
                for dst, src, s in qa:
                    sy.dma_start(out=dst, in_=src).then_inc(s, 16)
                sy.wait_ge(s_cp0, 1)
                sy.dma_start(out=out_d[0],
                             in_=outs_sb[0][:]).then_inc(s_od[0], 16)

            @block.scalar
            def _(sc):
                for dst, src, s in qb:
                    sc.dma_start(out=dst, in_=src).then_inc(s, 16)
                sc.wait_ge(s_c1v, 1)
                sc.dma_start(out=out_d[1],
                             in_=outs_sb[1][:]).then_inc(s_od[1], 16)

            @block.vector
            def _(v):
                v.memset(ones_sb[:], 1.0)
                v.drain()
                v.sem_inc(s_v, 1)
                v.wait_ge(s_ps, 1)
                v.tensor_copy(outs_sb[0][:], psums[0][:]).then_inc(s_cp0, 1)
                v.wait_ge(s_ps, 2)
                v.tensor_copy(outs_sb[1][:], psums[1][:]).then_inc(s_c1v, 1)

            if npair:
                # CSC via gpsimd's own SWDGE: its descriptor-gen runs in
                # parallel with the two HWDGE rings'. The warm memset comes
                # first so the PE can start warmups ~0.5us into the block.
                @block.gpsimd
                def _(g):
                    g.memset(warm_sb[:], 0.125)
                    g.drain()
                    g.sem_inc(s_w, 1)
                    g.dma_start(out=idx_sb[:], in_=idx_d[:]).then_inc(s_ci[0], 16)
                    g.dma_start(out=val_sb[:], in_=val_d[:]).then_inc(s_cv[0], 16)
                    g.load_library(library_config.local_scatter)
                    g.wait_ge(s_ci[0], 16)
                    g.wait_ge(s_cv[0], 16)
                    for j in range(npair):
                        g.local_scatter(
                            at_sb[:, j],
                            val_sb[:, j],
                            idx_sb[:, j],
                            channels=128,
                            num_elems=2 * O_SH,
                            num_idxs=wpad,
                        ).then_inc(s_g, 1)

            @block.tensor
            def _(te):
                # back-to-back warmups: lift the HAM clock gate (~3.4us of
                # continuous PE busy) while the first real tiles stream in
                te.wait_ge(s_w, 1)
                for _ in range(N_WARM):
                    te.matmul(ps_warm[:], warm_sb[:, :128], warm_sb[:, 128:],
                              start=True, stop=True, skip_group_check=True)

                seen = set()
                g_thr = 0

                def wait_once(s):
                    if s.name not in seen:
                        te.wait_ge(s, 16)
                        seen.add(s.name)

                def rhs_of(k):
                    info = kinfo[k]
                    if info[0] == "d":
                        slot = info[1]
                        wait_once(s_atd[chunk_of(atd_chunks, slot)])
                        return atd_sb[:, slot, :]
                    _, j, h = info
                    nonlocal g_thr
                    if j + 1 > g_thr:
                        te.wait_ge(s_g, j + 1)
                        g_thr = j + 1
                    return at_sb[:, j, h, :]

                def mm(k, i, stop=False):
                    m = te.matmul(psums[i][:],
                                  inT_sb[:, k, 128 * i:128 * (i + 1)],
                                  rhs_of(k), start=(k == 0), stop=stop)
                    if stop:
                        m.then_inc(s_ps, 1)

                for k in range(NT - STAG):
                    wait_once(s_in[chunk_of(in_chunks, k)])
                    for i in range(NB):
                        mm(k, i)
                    if k == 1:
                        # bias lands in psum via ones^T @ bias once the tiny
                        # DMA is in (off the critical path)
                        te.wait_ge(s_bias, 16)
                        for i in range(NB):
                            te.matmul(psums[i][:], ones_sb[:], bias_sb[:],
                                      start=False, stop=False)
                # psum0 finishes STAG tiles early so copy0 + out0 overlap
                for k in range(NT - STAG, NT):
                    wait_once(s_in[chunk_of(in_chunks, k)])
                    mm(k, 0, stop=(k == NT - 1))
                for k in range(NT - STAG, NT):
                    mm(k, 1, stop=(k == NT - 1))

        # after the work block's all-engine barrier, recycle semaphores so
        # the next execution of this NEFF starts from zero
        all_sems = ([s_bias, s_g, s_v, s_w, s_ps, s_cp0, s_c1v]
                    + s_in + s_atd + s_ci + s_cv)
        with nc.Block() as block2:

            @block2.sync
            def _(sy):
                for s in all_sems:
                    sy.sem_clear(s)

    nc.compile()
    return nc


def _build_program(wpad: int, npair: int):
    key = (wpad, npair)
    if key not in _prog_cache:
        _prog_cache[key] = _build_program_raw(wpad, npair)
    return _prog_cache[key]


def _prepare(input, condensed_weight, input_mask, bias, npair=NPAIR):
    """Host-side repack: dedupe + CSC-bin the sparse weights, cast/transpose
    the activations, permute f-tiles into consumption order."""
    nd = NT - 2 * npair
    korder, kinfo = _schedule(npair)

    # input^T bf16 tiled [128f, NT, B] in consumption order:
    # v[p, k, b] = input[b, 128*korder[k] + p]
    inT = np.ascontiguousarray(
        input.astype(_BF16).T.reshape(NT, 128, B)[korder].transpose(1, 0, 2))

    # dedupe (o, f) pairs, summing weights in f64
    o_idx = np.repeat(np.arange(OUT_F, dtype=np.int64), K)
    f_idx = input_mask.ravel().astype(np.int64)
    w = condensed_weight.ravel()
    key = (o_idx << 12) | f_idx
    uk, inv = np.unique(key, return_inverse=True)
    sums = np.bincount(inv, weights=w.astype(np.float64))
    o_u = (uk >> 12).astype(np.int64)
    f_u = (uk & (IN_F - 1)).astype(np.int64)
    v_u = sums.astype(np.float32)

    core = o_u // O_SH
    o_loc = o_u % O_SH
    t_id = f_u // 128
    p_f = f_u % 128

    dense_m = t_id < nd
    if nd:
        # atd slot s holds the s-th dense tile in consumption order, which
        # by construction of _schedule is orig tile s itself
        atd = np.zeros((N_CORES, 128, nd, O_SH), dtype=_BF16)
        atd[core[dense_m], p_f[dense_m], t_id[dense_m], o_loc[dense_m]] = \
            v_u[dense_m]

    wpad = 2
    if npair:
        sm = ~dense_m
        ts = t_id[sm] - nd
        s_core, s_p, s_o, s_v = core[sm], p_f[sm], o_loc[sm], v_u[sm]
        s_pair = ts // 2
        # index within the merged pair tile: second tile offset by O_SH
        s_idx = s_o + O_SH * (ts % 2)
        # rank of each entry within its (core, partition, pair) group
        g = (s_core * 128 + s_p) * npair + s_pair
        order = np.argsort(g, kind="stable")
        gs = g[order]
        change = np.r_[True, gs[1:] != gs[:-1]]
        seg_start = np.flatnonzero(change)
        seg_id = np.cumsum(change) - 1
        rank = np.arange(gs.size) - seg_start[seg_id]

        maxc = int(rank.max()) + 1 if gs.size else 0
        wpad = max(2, (maxc + 1) // 2 * 2)

        idx_arr = np.full((N_CORES, 128, npair, wpad), -1, dtype=np.int16)
        val_arr = np.zeros((N_CORES, 128, npair, wpad), dtype=_BF16)
        idx_arr[s_core[order], s_p[order], s_pair[order], rank] = \
            s_idx[order].astype(np.int16)
        val_arr[s_core[order], s_p[order], s_pair[order], rank] = s_v[order]

    in_maps = []
    for c in range(N_CORES):
        m = {
            "inT": inT,
            "bias": np.ascontiguousarray(
                bias[c * O_SH:(c + 1) * O_SH].reshape(1, O_SH)
            ).astype(_BF16),
        }
        if npair:
            m["cscidx"] = np.ascontiguousarray(idx_arr[c])
            m["cscval"] = np.ascontiguousarray(val_arr[c])
        if nd:
            m["atd"] = np.ascontiguousarray(atd[c])
        in_maps.append(m)
    return in_maps, wpad


def kernel(input, condensed_weight, input_mask, bias,
           _run_kwargs=None, _res_box=None):
    """Full inputs in, full output out. Shards over 8 NeuronCores inside."""
    from concourse.bass_utils import run_bass_kernel_spmd

    in_maps, wpad = _prepare(
        np.asarray(input), np.asarray(condensed_weight),
        np.asarray(input_mask), np.asarray(bias))
    nc = _build_program(wpad, NPAIR)

    res = run_bass_kernel_spmd(nc, in_maps, list(range(N_CORES)),
                               **(_run_kwargs or {}))
    if _res_box is not None:
        _res_box["results"] = res

    out = np.concatenate(
        [np.asarray(res.results[c]["out"]).reshape(B, O_SH)
         for c in range(N_CORES)], axis=1)
    return out.astype(np.float32)


# revision 28
# speedup vs baseline: 1.0539x; 1.0539x over previous
"""CondensedLinearFineGrained on 8 TRN2 NeuronCores.

Math: out[b,o] = sum_k W[o,k] * input[b, mask[o,k]] + bias[o]
with B=256, IN_F=4096, OUT_F=4096, K=256.

Strategy
--------
Reformulate as a dense matmul:  out = input @ A^T + bias  where
A[o,f] = sum_{k: mask[o,k]==f} W[o,k]  (duplicates within a row are summed).
The dense reformulation does 16x the MACs of the gather form but runs on the
128x128 systolic array (~13.8us/core bf16); the gather form would push 268M
elements through vector/gpsimd (hundreds of us).

Sharding: output neurons, 512 per core. Per core the A^T f-tiles
[128f x 512o] bf16 come from two producers running in parallel:
  - N_DENSE tiles pre-densified on the host, bulk-DMA'd (131KB/tile).
  - NPAIR*2 tiles built on-device by gpsimd local_scatter from host-packed
    CSC (per-feature (o,weight) lists, deduped, -1-padded, int16 idx),
    two tiles per scatter (~1.34us/pair, ~51KB CSC per pair).
The split is chosen so the DMA stream (~335GB/s measured) and the gpsimd
scatter pipeline both finish just under the PE's own span (64 matmuls of
[128bx128f]@[128f x 512o] ~= 216ns each warm).

Schedule notes (from perfetto traces):
  - exec_time is measured from the first engine instruction to the last
    sequencer drain, so the tail matters as much as the stream: semaphores
    are recycled inline on their waiter engines (no second block), psum0 is
    finished STAG tiles early so its copy+out-DMA overlap the remaining
    matmuls, and copy of psum1 is split across DVE and Act.
  - The PE HAM clock gate needs ~3.4us of *continuous* PE busy time to lift
    the 1.2GHz->2.4GHz throttle, so the PE stream opens with back-to-back
    warmup matmuls on uninitialized SBUF with no semaphore waits, sized to
    bridge until the first real tile's data lands.
  - Tiles are consumed in an order that interleaves scattered pairs (gpsimd
    paced, 1.34us/pair) with dense tiles so neither producer stalls the PE;
    the host permutes inT/atd into consumption order so DMA chunks stay
    contiguous.
"""

import numpy as np
import ml_dtypes

B = 256
IN_F = 4096
OUT_F = 4096
K = 256
N_CORES = 8
O_SH = OUT_F // N_CORES  # 512 output rows per core
NT = IN_F // 128         # 32 feature tiles
NB = B // 128            # 2 batch tiles

NPAIR = 8        # scatter pairs (2 f-tiles each) built on-device
N_WARM = 9       # back-to-back PE warmups to lift the HAM clock gate
STAG = 4         # psum0 finishes STAG tiles before psum1 (tail overlap)

_BF16 = ml_dtypes.bfloat16

_prog_cache = {}


def _schedule(npair: int):
    """Consumption-order schedule shared by host packing and the program.

    Returns (korder, kinfo) where korder[k] = original f-tile id consumed
    at step k, and kinfo[k] = ('d', atd_slot) or ('s', pair, half).
    Dense tiles are orig f-tiles [0, nd); pair j is orig tiles
    (nd+2j, nd+2j+1). Order: 2 dense up front, then each pair followed by
    one dense filler (pair production is slower than PE consumption), then
    the remaining dense tiles.
    """
    nd = NT - 2 * npair
    korder, kinfo = [], []
    slot = 0

    def dense(n):
        nonlocal slot
        for _ in range(n):
            if slot < nd:
                korder.append(slot)
                kinfo.append(("d", slot))
                slot += 1

    dense(2)
    for j in range(npair):
        korder.append(nd + 2 * j)
        kinfo.append(("s", j, 0))
        korder.append(nd + 2 * j + 1)
        kinfo.append(("s", j, 1))
        dense(1)
    dense(nd)
    assert len(korder) == NT
    return korder, kinfo


def _chunk_plan(npair: int):
    """(inT chunks, atd chunks, csc chunks) as lists of (start, end)."""
    nd = NT - 2 * npair

    def chunks(total, sizes):
        out, p, i = [], 0, 0
        while p < total:
            s = min(sizes[min(i, len(sizes) - 1)], total - p)
            out.append((p, p + s))
            p += s
            i += 1
        return out

    # Each dma_start costs ~0.7us of serialized descriptor-gen on its
    # issuing sequencer, so chunks are few and grow toward the tail.
    in_chunks = chunks(NT, [3, 5, 6, 7, 7])
    atd_chunks = chunks(nd, [2, 2, 2, 2, 2, 3])
    csc_chunks = [(0, npair)]
    return in_chunks, atd_chunks, csc_chunks


def _build_program_raw(wpad: int, npair: int):
    """Hand-scheduled SPMD program: explicit per-engine streams + semaphores."""
    from contextlib import ExitStack
    from concourse import bacc, mybir, library_config

    nd = NT - 2 * npair
    korder, kinfo = _schedule(npair)
    in_chunks, atd_chunks, csc_chunks = _chunk_plan(npair)

    def chunk_of(chunks, t):
        for c, (c0, c1) in enumerate(chunks):
            if c0 <= t < c1:
                return c
        raise AssertionError

    nc = bacc.Bacc("TRN2", target_bir_lowering=False, debug=False)
    dt = mybir.dt

    inT_d = nc.dram_tensor("inT", [128, NT, B], dt.bfloat16, kind="ExternalInput")
    bias_d = nc.dram_tensor("bias", [1, O_SH], dt.bfloat16, kind="ExternalInput")
    if npair:
        idx_d = nc.dram_tensor("cscidx", [128, npair, wpad], dt.int16,
                               kind="ExternalInput")
        val_d = nc.dram_tensor("cscval", [128, npair, wpad], dt.bfloat16,
                               kind="ExternalInput")
    if nd:
        atd_d = nc.dram_tensor("atd", [128, nd, O_SH], dt.bfloat16,
                               kind="ExternalInput")
    out_d = nc.dram_tensor("out", [NB, 128, O_SH], dt.float32,
                           kind="ExternalOutput")

    inT_sb = nc.alloc_sbuf_tensor("inT_sb", [128, NT, B], dt.bfloat16).ap()
    bias_sb = nc.alloc_sbuf_tensor("bias_sb", [1, O_SH], dt.bfloat16).ap()
    ones_sb = nc.alloc_sbuf_tensor("ones_sb", [1, 128], dt.bfloat16).ap()
    warm_sb = nc.alloc_sbuf_tensor("warm_sb", [128, 128 + O_SH],
                                   dt.bfloat16).ap()
    if npair:
        idx_sb = nc.alloc_sbuf_tensor("idx_sb", [128, npair, wpad],
                                      dt.int16).ap()
        val_sb = nc.alloc_sbuf_tensor("val_sb", [128, npair, wpad],
                                      dt.bfloat16).ap()
        at_sb = nc.alloc_sbuf_tensor("at_sb", [128, npair, 2, O_SH],
                                     dt.bfloat16).ap()
    if nd:
        atd_sb = nc.alloc_sbuf_tensor("atd_sb", [128, nd, O_SH],
                                      dt.bfloat16).ap()
    outs_sb = [nc.alloc_sbuf_tensor(f"out_sb{i}", [128, O_SH], dt.float32).ap()
               for i in range(NB)]

    psums = [nc.alloc_psum_tensor(f"ps{i}", [128, O_SH], dt.float32).ap()
             for i in range(NB)]
    ps_warm = nc.alloc_psum_tensor("ps_warm", [128, O_SH], dt.float32).ap()

    with ExitStack() as ctx:
        sem = lambda name: ctx.enter_context(nc.semaphore(name))
        # One semaphore per DMA: sub-transfers of back-to-back DMAs on one
        # queue can complete out of order, so prefix thresholds on a shared
        # semaphore would be unsound.
        s_bias = sem("s_bias")
        s_in = [sem(f"s_in{c}") for c in range(len(in_chunks))]
        s_atd = [sem(f"s_atd{c}") for c in range(len(atd_chunks))] if nd else []
        s_ci = [sem(f"s_ci{c}") for c in range(len(csc_chunks))] if npair else []
        s_cv = [sem(f"s_cv{c}") for c in range(len(csc_chunks))] if npair else []
        # out-DMA completion sems: incremented (the BIR verifier requires a
        # sem update on every DMA) but never waited in-kernel — the
        # runtime's queue drain covers output completion. Never cleared:
        # nothing depends on their value.
        s_od = [sem(f"s_od{i}") for i in range(NB)]
        s_g = sem("s_g")      # scatter pairs published
        s_v = sem("s_v")      # ones_sb ready
        s_w = sem("s_w")      # warm_sb ready
        s_ps = sem("s_ps")    # PE accumulation done per psum
        s_cp0 = sem("s_cp0")  # psum0->sbuf copy done
        s_c1v = sem("s_c1v")  # psum1 copy done

        with nc.Block() as block:

            # The input feed is interleaved across the two HWDGE rings in
            # PE/gpsimd consumption order, greedily byte-balanced so neither
            # ring starves a consumer. CSC chunks ride early (gpsimd's
            # scatter pipeline is the slowest producer); atd/inT chunks
            # follow the tile consumption order (host pre-permuted).
            feed = []  # (dst, src, sem, bytes)

            def add(dst, src, s, nbytes):
                feed.append((dst, src, s, nbytes))

            ic = lambda c: in_chunks[c]
            ac = lambda c: atd_chunks[c]
            cc = lambda c: csc_chunks[c]

            add(bias_sb[:], bias_d[:], s_bias, O_SH * 2)
            # CSC first: the gpsimd scatter pipeline (1.34us/pair, serial)
            # is the longest producer chain, so its data must land first
            add(idx_sb[:], idx_d[:], s_ci[0], npair * wpad)
            add(val_sb[:], val_d[:], s_cv[0], npair * wpad)
            for c in range(max(len(in_chunks), len(atd_chunks))):
                if nd and c < len(atd_chunks):
                    c0, c1 = ac(c)
                    add(atd_sb[:, c0:c1, :], atd_d[:, c0:c1, :], s_atd[c],
                        (c1 - c0) * O_SH * 2)
                if c < len(in_chunks):
                    c0, c1 = ic(c)
                    add(inT_sb[:, c0:c1, :], inT_d[:, c0:c1, :], s_in[c],
                        (c1 - c0) * B * 2)

            qa, qb, ba, bb = [], [], 0, 0
            for dst, src, s, w in feed:
                if ba <= bb:
                    qa.append((dst, src, s)); ba += w
                else:
                    qb.append((dst, src, s)); bb += w

            @block.sync
            def _(sy):
                for dst, src, s in qa:
                    sy.dma_start(out=dst, in_=src).then_inc(s, 16)
                sy.wait_ge(s_cp0, 1)
                sy.dma_start(out=out_d[0],
                             in_=outs_sb[0][:]).then_inc(s_od[0], 16)
                sy.sem_clear(s_cp0)

            @block.scalar
            def _(sc):
                for dst, src, s in qb:
                    sc.dma_start(out=dst, in_=src).then_inc(s, 16)
                sc.wait_ge(s_c1v, 1)
                sc.dma_start(out=out_d[1],
                             in_=outs_sb[1][:]).then_inc(s_od[1], 16)
                sc.sem_clear(s_ps)
                sc.sem_clear(s_c1v)

            @block.vector
            def _(v):
                v.memset(ones_sb[:], 1.0)
                v.drain()
                v.sem_inc(s_v, 1)
                v.wait_ge(s_ps, 1)
                v.tensor_copy(outs_sb[0][:], psums[0][:]).then_inc(s_cp0, 1)
                v.wait_ge(s_ps, 2)
                v.tensor_copy(outs_sb[1][:], psums[1][:]).then_inc(s_c1v, 1)

            if npair:
                # warm memset first so the PE can start warmups ~0.5us into
                # the block; library load overlaps the CSC DMA
                @block.gpsimd
                def _(g):
                    g.memset(warm_sb[:], 0.125)
                    g.drain()
                    g.sem_inc(s_w, 1)
                    g.load_library(library_config.local_scatter)
                    g.wait_ge(s_ci[0], 16)
                    g.wait_ge(s_cv[0], 16)
                    for j in range(npair):
                        g.local_scatter(
                            at_sb[:, j],
                            val_sb[:, j],
                            idx_sb[:, j],
                            channels=128,
                            num_elems=2 * O_SH,
                            num_idxs=wpad,
                        ).then_inc(s_g, 1)
                    g.sem_clear(s_ci[0])
                    g.sem_clear(s_cv[0])

            @block.tensor
            def _(te):
                # back-to-back warmups: lift the HAM clock gate (~3.4us of
                # continuous PE busy) while the first real tiles stream in
                te.wait_ge(s_w, 1)
                for _ in range(N_WARM):
                    te.matmul(ps_warm[:], warm_sb[:, :128], warm_sb[:, 128:],
                              start=True, stop=True, skip_group_check=True)

                seen = set()
                g_thr = 0

                def wait_once(s):
                    if s.name not in seen:
                        te.wait_ge(s, 16)
                        seen.add(s.name)

                def rhs_of(k):
                    info = kinfo[k]
                    if info[0] == "d":
                        slot = info[1]
                        wait_once(s_atd[chunk_of(atd_chunks, slot)])
                        return atd_sb[:, slot, :]
                    _, j, h = info
                    nonlocal g_thr
                    if j + 1 > g_thr:
                        te.wait_ge(s_g, j + 1)
                        g_thr = j + 1
                    return at_sb[:, j, h, :]

                def mm(k, i, stop=False):
                    m = te.matmul(psums[i][:],
                                  inT_sb[:, k, 128 * i:128 * (i + 1)],
                                  rhs_of(k), start=(k == 0), stop=stop)
                    if stop:
                        m.then_inc(s_ps, 1)

                for k in range(NT - STAG):
                    wait_once(s_in[chunk_of(in_chunks, k)])
                    for i in range(NB):
                        mm(k, i)
                    if k == 1:
                        # bias lands in psum via ones^T @ bias once the tiny
                        # DMA + memset are in (off the critical path)
                        te.wait_ge(s_v, 1)
                        te.wait_ge(s_bias, 16)
                        for i in range(NB):
                            te.matmul(psums[i][:], ones_sb[:], bias_sb[:],
                                      start=False, stop=False)
                # psum0 finishes STAG tiles early so copy0 + out0 overlap
                for k in range(NT - STAG, NT):
                    wait_once(s_in[chunk_of(in_chunks, k)])
                    mm(k, 0, stop=(k == NT - 1))
                for k in range(NT - STAG, NT):
                    mm(k, 1, stop=(k == NT - 1))

                # recycle semaphores inline (each on its waiter engine,
                # after that engine's final wait) so the next execution of
                # this NEFF starts from zero without a second block+barrier
                te.sem_clear(s_v)
                te.sem_clear(s_w)
                te.sem_clear(s_bias)
                te.sem_clear(s_g)
                for s in s_in:
                    te.sem_clear(s)
                for s in s_atd:
                    te.sem_clear(s)

    nc.compile()
    return nc


def _build_program(wpad: int, npair: int):
    key = (wpad, npair)
    if key not in _prog_cache:
        _prog_cache[key] = _build_program_raw(wpad, npair)
    return _prog_cache[key]


def _prepare(input, condensed_weight, input_mask, bias, npair=NPAIR):
    """Host-side repack: dedupe + CSC-bin the sparse weights, cast/transpose
    the activations, permute f-tiles into consumption order."""
    nd = NT - 2 * npair
    korder, kinfo = _schedule(npair)

    # input^T bf16 tiled [128f, NT, B] in consumption order:
    # v[p, k, b] = input[b, 128*korder[k] + p]
    inT = np.ascontiguousarray(
        input.astype(_BF16).T.reshape(NT, 128, B)[korder].transpose(1, 0, 2))

    # dedupe (o, f) pairs, summing weights in f64
    o_idx = np.repeat(np.arange(OUT_F, dtype=np.int64), K)
    f_idx = input_mask.ravel().astype(np.int64)
    w = condensed_weight.ravel()
    key = (o_idx << 12) | f_idx
    uk, inv = np.unique(key, return_inverse=True)
    sums = np.bincount(inv, weights=w.astype(np.float64))
    o_u = (uk >> 12).astype(np.int64)
    f_u = (uk & (IN_F - 1)).astype(np.int64)
    v_u = sums.astype(np.float32)

    core = o_u // O_SH
    o_loc = o_u % O_SH
    t_id = f_u // 128
    p_f = f_u % 128

    dense_m = t_id < nd
    if nd:
        # atd slot s holds the s-th dense tile in consumption order, which
        # by construction of _schedule is orig tile s itself
        atd = np.zeros((N_CORES, 128, nd, O_SH), dtype=_BF16)
        atd[core[dense_m], p_f[dense_m], t_id[dense_m], o_loc[dense_m]] = \
            v_u[dense_m]

    wpad = 2
    if npair:
        sm = ~dense_m
        ts = t_id[sm] - nd
        s_core, s_p, s_o, s_v = core[sm], p_f[sm], o_loc[sm], v_u[sm]
        s_pair = ts // 2
        # index within the merged pair tile: second tile offset by O_SH
        s_idx = s_o + O_SH * (ts % 2)
        # rank of each entry within its (core, partition, pair) group
        g = (s_core * 128 + s_p) * npair + s_pair
        order = np.argsort(g, kind="stable")
        gs = g[order]
        change = np.r_[True, gs[1:] != gs[:-1]]
        seg_start = np.flatnonzero(change)
        seg_id = np.cumsum(change) - 1
        rank = np.arange(gs.size) - seg_start[seg_id]

        maxc = int(rank.max()) + 1 if gs.size else 0
        wpad = max(2, (maxc + 1) // 2 * 2)

        idx_arr = np.full((N_CORES, 128, npair, wpad), -1, dtype=np.int16)
        val_arr = np.zeros((N_CORES, 128, npair, wpad), dtype=_BF16)
        idx_arr[s_core[order], s_p[order], s_pair[order], rank] = \
            s_idx[order].astype(np.int16)
        val_arr[s_core[order], s_p[order], s_pair[order], rank] = s_v[order]

    in_maps = []
    for c in range(N_CORES):
        m = {
            "inT": inT,
            "bias": np.ascontiguousarray(
                bias[c * O_SH:(c + 1) * O_SH].reshape(1, O_SH)
            ).astype(_BF16),
        }
        if npair:
            m["cscidx"] = np.ascontiguousarray(idx_arr[c])
            m["cscval"] = np.ascontiguousarray(val_arr[c])
        if nd:
            m["atd"] = np.ascontiguousarray(atd[c])
        in_maps.append(m)
    return in_maps, wpad


def kernel(input, condensed_weight, input_mask, bias,
           _run_kwargs=None, _res_box=None):
    """Full inputs in, full output out. Shards over 8 NeuronCores inside."""
    from concourse.bass_utils import run_bass_kernel_spmd

    in_maps, wpad = _prepare(
        np.asarray(input), np.asarray(condensed_weight),
        np.asarray(input_mask), np.asarray(bias))
    nc = _build_program(wpad, NPAIR)

    res = run_bass_kernel_spmd(nc, in_maps, list(range(N_CORES)),
                               **(_run_kwargs or {}))
    if _res_box is not None:
        _res_box["results"] = res

    out = np.concatenate(
        [np.asarray(res.results[c]["out"]).reshape(B, O_SH)
         for c in range(N_CORES)], axis=1)
    return out.astype(np.float32)
